# revision 11
# baseline (speedup 1.0000x reference)
"""Segment+causal masked attention with bias, TRN2 Bass kernel, 8 NeuronCores.

Reference computation (per batch b, head h):
    logits = q @ k.T * sm_scale + bias
    masked where NOT (same-segment AND causal) -> -inf
    out = softmax(logits) @ v

Sharding: head-parallel. Each of the 8 cores owns 2 heads x 2 batches = 4
(b,h) pairs and computes them independently (no collectives).

Device algorithm (per (b,h) pair, block-sparse over active 128x128 tiles
of the [key, query]-transposed score matrix):
    logitsT[k,q] = kT.T @ qT              (TensorE, bf16, PSUM f32)
    el = exp(logitsT)                     (ScalarE, one inst per 4-tile group)
    w  = el * ebT                         (VectorE, ebT = host-staged
                                           exp(bias) * mask, transposed)
    outU[q, 0:64] += w.T @ v ; outU[q,64] += w.T @ 1   (TensorE, PSUM accum;
                                           ones column = softmax denominator)
Host divides outU[:, :64] by outU[:, 64] at the end. The mask and the bias
are folded into one staged tensor (exp(b) zeroed where masked), and all
transposes are done on the host, so the device does no transposes, no
reductions and no max-subtraction (value range makes exp safe in f32/bf16).

Groups are 4 tiles (= full 512-f32 PSUM bank per head) packed across
output-block boundaries so every exp/mul runs at maximum width. eb is
shipped in multi-group chunks on the gpsimd (SWDGE) queue while qk/va/o
travel on the sync (HWDGE) queue; outputs are stored per 4-q-tile block as
soon as they are reduced, so the kernel tail is one small store + drain.
"""
import math

import numpy as np
import ml_dtypes

import sys

sys.path.insert(0, "/opt/trn_rl_repo")

import concourse.bass as bass  # noqa: E402
import concourse.tile as tile  # noqa: E402
from concourse import bacc, mybir  # noqa: E402
from concourse.bass_utils import run_bass_kernel_spmd  # noqa: E402

bf16 = ml_dtypes.bfloat16

B, S, H, C = 2, 2048, 16, 64
T = 128
NT = S // T  # 16 q/k tiles per sequence
NCORE = 8
HPC = H // NCORE  # heads per core
PAIRS = B * HPC  # (b, h_local) pairs per core; p -> batch = p // HPC
SM = 1.0 / math.sqrt(C)
GCAP = 4  # tiles per group per head (duo: 2 heads x 4 tiles)
OUT_BLK = 4  # q-tiles per PSUM output block ([128, 4*65] fits one bank)
CHUNK = 3  # groups per eb DMA chunk
VW = C + 1  # v width with ones column
HOIST = True  # move initial input DMAs ahead of the engine-preamble barrier


def _plan(m: np.ndarray):
    """Static schedule from segment ids.

    Returns (kstart, groups): kstart[b][i] = first active k-tile of q-tile i;
    groups[b] = list of tile groups, each a list of up to GCAP (i, j) tiles
    in i-major traversal order, packed across output-block boundaries so
    almost every group is full width.
    """
    kstart = []
    for b_ in range(B):
        mm = m[b_]
        segstart = np.searchsorted(mm, mm)
        kstart.append([int(segstart[i * T]) // T for i in range(NT)])

    groups = []
    for b_ in range(B):
        ks = kstart[b_]
        tiles = [(i, j) for i in range(NT) for j in range(ks[i], i + 1)]
        groups.append([tiles[c0:c0 + GCAP] for c0 in range(0, len(tiles), GCAP)])
    return kstart, groups


class _FastTailTile(tile.TileContext):
    """TileContext with a minimal kernel tail.

    The stock exit emits drain + all-engine butterfly + semaphore clears +
    second butterfly (~9-13us on silicon). For a single-execution NEFF it is
    enough that one engine waits until every tracked semaphore reaches its
    final value (which includes all DMA completions) and then clears the
    semaphores: executions are serialized by the runtime, so no cross-engine
    barrier is needed after the clear.
    """

    def _drain_and_barrier(self, tick_clock, wait_clock):
        drain_inst = self.nc.gpsimd.drain()
        wait_clock.add_sem_waits(
            drain_inst.ins, tile.ScopedClock({None: tick_clock.global_clock})
        )
        popped = self.nc._tile_sem_poison_stack.pop()
        assert popped is self._sem_poison
        self.nc.clear_and_free_semaphores(list(self.sems.allocated().values()))


def _build(kstart, groups):
    """Build the Bass graph.

    Software-pipelined stages: A (eb DMA + QK^T), B (exp + multiply),
    C (PV accumulate + per-block output store), emitted B(g-1), A(g),
    C(g-2) so the in-order PE always has QK work queued between PV batches.

    Duo execution: the core's two heads of batch b run concurrently -
    head A's QK^T matmuls on PE rows 0-63 into the first PSUM bank of the
    group's l tile, head B's on rows 64-127 into the second bank.
    """
    nc = bacc.Bacc("TRN2", target_bir_lowering=False, debug=False,
                   num_devices=NCORE)
    dt = mybir.dt
    # qt+kt merged, per duo: [kt-half0 | qt-half0 | kt-half1 | qt-half1]
    qk = nc.dram_tensor("qk", [2 * C, (PAIRS // 2) * 2 * S], dt.bfloat16, kind="ExternalInput").ap()
    va = nc.dram_tensor("va", [T, PAIRS * NT * VW], dt.bfloat16, kind="ExternalInput").ap()

    # flatten (duo, group) sequence; chunk eb DMA every CHUNK groups
    GL = []  # (duo, tiles, eb_offset_cols)
    eboff = 0
    for du in range(PAIRS // 2):
        for g in groups[du]:
            GL.append((du, g, eboff))
            eboff += 2 * len(g) * T
    n = len(GL)
    bounds = [0, min(2, n)]  # small first chunk so the first mul lands early
    while bounds[-1] < n:
        bounds.append(min(bounds[-1] + CHUNK, n))
    CH = []  # chunk -> (start col, n cols)
    chunk_of = [0] * n
    for ci in range(len(bounds) - 1):
        g0, g1 = bounds[ci], bounds[ci + 1]
        lo = GL[g0][2]
        hi = GL[g1][2] if g1 < n else eboff
        CH.append((lo, hi - lo))
        for g in range(g0, g1):
            chunk_of[g] = ci
    nch = len(CH)

    eb = nc.dram_tensor("eb", [T, eboff], dt.bfloat16, kind="ExternalInput").ap()
    o = nc.dram_tensor("o", [T, PAIRS, NT * VW], dt.bfloat16, kind="ExternalOutput").ap()
    CAP = GCAP * T

    with _FastTailTile(nc) as tc:
        with (
            tc.tile_pool(name="res", bufs=1) as res,
            tc.tile_pool(name="io", bufs=8) as io,
            tc.tile_pool(name="wk", bufs=6) as wk,
            tc.tile_pool(name="ob", bufs=4) as obp,
            tc.tile_pool(name="ops", bufs=2, space="PSUM") as ops,
            tc.tile_pool(name="lps", bufs=2, space="PSUM") as lps,
        ):
            # Warm the ScalarE Exp spline table during the DMA preamble:
            # walrus loads the ACT table set at the first ACTIVATE (~2.7us),
            # which otherwise lands on the first group's critical chain.
            hoist = []
            warm = res.tile([T, 1], dt.float32, tag="actwarm")
            hoist.append(nc.gpsimd.memset(warm[:], 0.0).ins)
            hoist.append(nc.scalar.activation(
                warm[:], warm[:], mybir.ActivationFunctionType.Exp).ins)

            qt_sb, va_sb = {}, {}
            st = {}  # g -> dict of live tiles
            chst = {}  # chunk -> eb tile
            o_ps = {}  # half -> current psum out block
            obt_cur = {}  # block -> staging tile

            def fetch_chunk(c):
                if c in chst or c >= nch:
                    return
                lo, cols = CH[c]
                ebt = io.tile([T, cols], dt.bfloat16, tag="eb", name=f"ebc{c}")
                # c0 rides the sync HWDGE ring; the next two (needed by
                # ~13us, before SWDGE wakes up) ride the scalar HWDGE ring
                # whose queue is idle until the first real exp; the rest use
                # the gpsimd SWDGE queue.
                if c == 0:
                    eng = nc.sync
                elif c <= 2:
                    eng = nc.scalar
                else:
                    eng = nc.gpsimd
                di = eng.dma_start(ebt[:], eb[:, lo:lo + cols])
                if c <= 2:
                    hoist.append(di.ins)
                chst[c] = (ebt, lo)

            def alloc_duo(du):
                pA, pB = 2 * du, 2 * du + 1
                qt_sb[du] = res.tile([2 * C, 2 * S], dt.bfloat16, tag=f"qk{du}", name=f"qk{du}")
                vduo = res.tile([T, 2 * NT * VW], dt.bfloat16, tag=f"va{du}", name=f"vad{du}")
                va_sb[pA] = vduo[:, 0:NT * VW]
                va_sb[pB] = vduo[:, NT * VW:2 * NT * VW]
                return vduo

            # prologue: the sync (HWDGE) queue carries the resident inputs
            # plus eb chunk0, ordered by first use, hoisted ahead of the
            # bass preamble; later eb chunks ride the gpsimd (SWDGE) queue
            # from the body (a hoisted SWDGE DMA would stall the Pool
            # preamble drain and with it the whole entry barrier).
            v0 = alloc_duo(0)
            v1 = alloc_duo(1)
            hoist.append(nc.sync.dma_start(qt_sb[0][:, 0:S], qk[:, 0:S]).ins)
            fetch_chunk(0)
            hoist.append(nc.sync.dma_start(v0[:], va[:, 0:2 * NT * VW]).ins)
            hoist.append(nc.scalar.dma_start(qt_sb[0][:, S:2 * S], qk[:, S:2 * S]).ins)
            hoist.append(nc.scalar.dma_start(qt_sb[1][:], qk[:, 2 * S:4 * S]).ins)
            hoist.append(nc.sync.dma_start(v1[:], va[:, 2 * NT * VW:4 * NT * VW]).ins)
            for _c in range(1, nch):
                fetch_chunk(_c)

            HS = S // 2

            def ktc(s0):
                return s0 if s0 < HS else s0 + HS

            def qtc(s0):
                return s0 + HS if s0 < HS else s0 + 2 * HS

            def stage_a(g):
                du, tg, off = GL[g]
                cols = len(tg) * T
                l_ps = lps.tile([T, 2 * CAP], dt.float32, tag="l", name=f"l{g}")
                for idx, (i, j) in enumerate(tg):
                    for h, c0 in ((0, 0), (C, CAP)):
                        nc.tensor.matmul(
                            l_ps[:, c0 + idx * T:c0 + (idx + 1) * T],
                            qt_sb[du][h:h + C, ktc(j * T):ktc(j * T) + T],
                            qt_sb[du][h:h + C, qtc(i * T):qtc(i * T) + T],
                            start=True, stop=True, skip_group_check=True,
                        )
                ebt, base = chst[chunk_of[g]]
                st[g] = dict(eb=ebt[:, off - base:off - base + 2 * cols], l=l_ps)

            def stage_b(g):
                du, tg, off = GL[g]
                cols = len(tg) * T
                el_sb = wk.tile([T, 2 * CAP], dt.bfloat16, tag="el", name=f"el{g}")
                nc.scalar.activation(el_sb[:, 0:CAP + cols],
                                     st[g]["l"][:, 0:CAP + cols],
                                     mybir.ActivationFunctionType.Exp)
                w_sb = wk.tile([T, 2 * cols], dt.bfloat16, tag="w", name=f"w{g}")
                if cols == CAP:
                    nc.vector.tensor_mul(w_sb[:], el_sb[:], st[g]["eb"])
                else:
                    nc.vector.tensor_mul(w_sb[:, 0:cols], el_sb[:, 0:cols],
                                         st[g]["eb"][:, 0:cols])
                    nc.vector.tensor_mul(w_sb[:, cols:2 * cols],
                                         el_sb[:, CAP:CAP + cols],
                                         st[g]["eb"][:, cols:2 * cols])
                st[g]["w"] = w_sb

            def stage_c(g):
                du, tg, off = GL[g]
                ks = kstart[du]
                w_sb = st[g]["w"]
                cols = len(tg) * T
                for half, p in ((0, 2 * du), (1, 2 * du + 1)):
                    for idx, (i, j) in enumerate(tg):
                        if j == ks[i] and i % OUT_BLK == 0:
                            o_ps[half] = ops.tile([T, OUT_BLK * VW], dt.float32,
                                                  tag=f"o{half}", name=f"o{half}_{g}_{i}")
                        t_ = i % OUT_BLK
                        nc.tensor.matmul(
                            o_ps[half][:, t_ * VW:(t_ + 1) * VW],
                            w_sb[:, half * cols + idx * T:half * cols + (idx + 1) * T],
                            va_sb[p][:, j * VW:(j + 1) * VW],
                            start=(j == ks[i]), stop=(j == i),
                            skip_group_check=True,
                        )
                        if j == i and i % OUT_BLK == OUT_BLK - 1:
                            blk = i // OUT_BLK
                            if half == 0:
                                obt_cur[blk] = obp.tile(
                                    [T, 2 * OUT_BLK * VW], dt.bfloat16,
                                    tag="obt", name=f"ob{g}_{i}")
                            obt = obt_cur[blk]
                            bw = OUT_BLK * VW
                            nc.vector.tensor_copy(
                                obt[:, half * bw:(half + 1) * bw], o_ps[half][:])
                            if half == 1:
                                c0 = blk * bw
                                nc.sync.dma_start(
                                    o[:, 2 * du:2 * du + 2, c0:c0 + bw], obt[:])
                del st[g]

            for g in range(n + 2):
                if 0 <= g - 1 < n:
                    stage_b(g - 1)
                if g < n:
                    stage_a(g)
                if g - 2 >= 0:
                    stage_c(g - 2)
    if HOIST:
        f = nc.m.functions[0]
        movable = [i for i in hoist
                   if not (i.sync_info and i.sync_info.on_wait)]
        names = {i.name for i in movable}
        for bb in f.blocks[1:]:
            kept = [i for i in bb.instructions if i.name not in names]
            if len(kept) != len(bb.instructions):
                bb.instructions = kept
        b0 = f.blocks[0]
        cur = b0.instructions
        b0.instructions = cur[:1] + movable + cur[1:]
    nc.compile()
    return nc


def _stage_inputs(q, k, v, b, m, groups):
    """Build per-core in_maps (host-side transposes, exp(bias)*mask, packing)."""
    ebtot = 2 * sum(len(g) for pg in groups for g in pg)
    masks = []
    for b_ in range(B):
        seg = m[b_][:, None] == m[b_][None, :]
        causal = np.tri(S, S, 0, dtype=bool)
        masks.append(seg & causal)

    ones = np.ones((S, 1), np.float32)
    in_maps = []
    for core in range(NCORE):
        qk = np.empty((2 * C, (PAIRS // 2) * 2 * S), bf16)
        HS = S // 2
        va = np.empty((T, PAIRS * NT * VW), bf16)
        ebp = np.empty((T, ebtot * T), bf16)
        E = {}
        for p in range(PAIRS):
            b_, h = p // HPC, HPC * core + p % HPC
            du, half = p // 2, p % 2
            base = du * 2 * S
            qT = (q[b_, :, h, :].T * SM).astype(bf16)
            kT = k[b_, :, h, :].T.astype(bf16)
            r0, r1 = half * C, (half + 1) * C
            qk[r0:r1, base:base + HS] = kT[:, 0:HS]
            qk[r0:r1, base + HS:base + 2 * HS] = qT[:, 0:HS]
            qk[r0:r1, base + 2 * HS:base + 3 * HS] = kT[:, HS:S]
            qk[r0:r1, base + 3 * HS:base + 4 * HS] = qT[:, HS:S]
            vv = np.concatenate([v[b_, :, h, :], ones], 1).astype(bf16)
            va[:, p * NT * VW:(p + 1) * NT * VW] = (
                vv.reshape(NT, T, VW).transpose(1, 0, 2).reshape(T, NT * VW))
            E[p] = np.exp(b[b_, h].astype(np.float32))
        eboff = 0
        for du in range(PAIRS // 2):
            Mk = masks[du]
            for g in groups[du]:
                for p in (2 * du, 2 * du + 1):
                    for (i, j) in g:
                        blk = np.where(Mk[i * T:(i + 1) * T, j * T:(j + 1) * T].T,
                                       E[p][i * T:(i + 1) * T, j * T:(j + 1) * T].T, 0.0)
                        ebp[:, eboff:eboff + T] = blk.astype(bf16)
                        eboff += T
        assert eboff == ebtot * T
        in_maps.append({"qk": qk, "va": va, "eb": ebp})
    return in_maps


def _unstage(results):
    """results[c]["o"] [T, PAIRS*NT*VW] f32 -> out [B, S, H, C] f32."""
    out = np.empty((B, S, H, C), np.float32)
    for core in range(NCORE):
        oc = np.asarray(results[core]["o"]).astype(np.float32).reshape(T, PAIRS, NT * VW)
        for p in range(PAIRS):
            b_, h = p // HPC, HPC * core + p % HPC
            blk = oc[:, p, :].reshape(T, NT, VW)
            blk = blk.transpose(1, 0, 2).reshape(S, VW)
            out[b_, :, h, :] = blk[:, :C] / blk[:, C:]
    return out


_CACHE = {}


def _get_nc(groups_key, kstart, groups):
    if groups_key not in _CACHE:
        _CACHE[groups_key] = _build(kstart, groups)
    return _CACHE[groups_key]


def kernel(q, k, v, b, m, _trace=False, _trace_cores=None):
    q = np.asarray(q, np.float32)
    k = np.asarray(k, np.float32)
    v = np.asarray(v, np.float32)
    b = np.asarray(b, np.float32)
    m = np.asarray(m)
    kstart, groups = _plan(m)
    groups_key = str(groups)
    nc = _get_nc(groups_key, kstart, groups)
    in_maps = _stage_inputs(q, k, v, b, m, groups)
    res = None
    for attempt in range(3):
        try:
            res = run_bass_kernel_spmd(nc, in_maps, core_ids=list(range(NCORE)),
                                       trace=_trace, trace_cores=_trace_cores)
            break
        except Exception:
            if attempt == 2:
                raise
    out = _unstage(res.results)
    kernel.last_results = res
    return out


if __name__ == "__main__":
    rng = np.random.default_rng(0)
    q = rng.standard_normal((B, S, H, C), np.float32)
    k = rng.standard_normal((B, S, H, C), np.float32)
    v = rng.standard_normal((B, S, H, C), np.float32)
    bb = rng.standard_normal((B, H, S, S), np.float32)
    mm = np.sort(rng.integers(0, 4, (B, S)).astype(np.int32), -1)
    o = kernel(q, k, v, bb, mm)
    print("kernel ran, out shape", o.shape, "finite:", np.isfinite(o).all())


# revision 12
# speedup vs baseline: 1.0659x; 1.0659x over previous
"""Segment+causal masked attention with bias, TRN2 Bass kernel, 8 NeuronCores.

Reference computation (per batch b, head h):
    logits = q @ k.T * sm_scale + bias
    masked where NOT (same-segment AND causal) -> -inf
    out = softmax(logits) @ v

Sharding: head-parallel. Each of the 8 cores owns 2 heads x 2 batches = 4
(b,h) pairs and computes them independently (no collectives).

Device algorithm (per (b,h) pair, block-sparse over active 128x128 tiles
of the [key, query]-transposed score matrix):
    logitsT[k,q] = kT.T @ qT              (TensorE, bf16, PSUM f32)
    el = exp(logitsT)                     (ScalarE, one inst per 4-tile group)
    w  = el * ebT                         (VectorE, ebT = host-staged
                                           exp(bias) * mask, transposed)
    outU[q, 0:64] += w.T @ v ; outU[q,64] += w.T @ 1   (TensorE, PSUM accum;
                                           ones column = softmax denominator)
Host divides outU[:, :64] by outU[:, 64] at the end. The mask and the bias
are folded into one staged tensor (exp(b) zeroed where masked), and all
transposes are done on the host, so the device does no transposes, no
reductions and no max-subtraction (value range makes exp safe in f32/bf16).

Groups are 4 tiles (= full 512-f32 PSUM bank per head) packed across
output-block boundaries so every exp/mul runs at maximum width. eb is
shipped in multi-group chunks on the gpsimd (SWDGE) queue while qk/va/o
travel on the sync (HWDGE) queue; outputs are stored per 4-q-tile block as
soon as they are reduced, so the kernel tail is one small store + drain.
"""
import math

import numpy as np
import ml_dtypes

import sys

sys.path.insert(0, "/opt/trn_rl_repo")

import concourse.bass as bass  # noqa: E402
import concourse.tile as tile  # noqa: E402
from concourse import bacc, mybir  # noqa: E402
from concourse.bass_utils import run_bass_kernel_spmd  # noqa: E402

bf16 = ml_dtypes.bfloat16

B, S, H, C = 2, 2048, 16, 64
T = 128
NT = S // T  # 16 q/k tiles per sequence
NCORE = 8
HPC = H // NCORE  # heads per core
PAIRS = B * HPC  # (b, h_local) pairs per core; p -> batch = p // HPC
SM = 1.0 / math.sqrt(C)
GCAP = 4  # tiles per group per head (duo: 2 heads x 4 tiles)
OUT_BLK = 4  # q-tiles per PSUM output block ([128, 4*65] fits one bank)
CHUNK = 3  # groups per eb DMA chunk
VW = C + 1  # v width with ones column
HOIST = True  # move initial input DMAs ahead of the engine-preamble barrier


def _plan(m: np.ndarray):
    """Static schedule from segment ids.

    Returns (kstart, groups): kstart[b][i] = first active k-tile of q-tile i;
    groups[b] = list of tile groups, each a list of up to GCAP (i, j) tiles
    in i-major traversal order, packed across output-block boundaries so
    almost every group is full width.
    """
    kstart = []
    for b_ in range(B):
        mm = m[b_]
        segstart = np.searchsorted(mm, mm)
        kstart.append([int(segstart[i * T]) // T for i in range(NT)])

    groups = []
    for b_ in range(B):
        ks = kstart[b_]
        tiles = [(i, j) for i in range(NT) for j in range(ks[i], i + 1)]
        groups.append([tiles[c0:c0 + GCAP] for c0 in range(0, len(tiles), GCAP)])
    return kstart, groups


class _FastTailTile(tile.TileContext):
    """TileContext with a minimal kernel tail.

    The stock exit emits drain + all-engine butterfly + semaphore clears +
    second butterfly (~9-13us on silicon). For a single-execution NEFF it is
    enough that one engine waits until every tracked semaphore reaches its
    final value (which includes all DMA completions) and then clears the
    semaphores: executions are serialized by the runtime, so no cross-engine
    barrier is needed after the clear.
    """

    def _drain_and_barrier(self, tick_clock, wait_clock):
        drain_inst = self.nc.gpsimd.drain()
        wait_clock.add_sem_waits(
            drain_inst.ins, tile.ScopedClock({None: tick_clock.global_clock})
        )
        popped = self.nc._tile_sem_poison_stack.pop()
        assert popped is self._sem_poison
        self.nc.clear_and_free_semaphores(list(self.sems.allocated().values()))


def _build(kstart, groups):
    """Build the Bass graph.

    Software-pipelined stages: A (eb DMA + QK^T), B (exp + multiply),
    C (PV accumulate + per-block output store), emitted B(g-1), A(g),
    C(g-2) so the in-order PE always has QK work queued between PV batches.

    Duo execution: the core's two heads of batch b run concurrently -
    head A's QK^T matmuls on PE rows 0-63 into the first PSUM bank of the
    group's l tile, head B's on rows 64-127 into the second bank.
    """
    nc = bacc.Bacc("TRN2", target_bir_lowering=False, debug=False,
                   num_devices=NCORE)
    dt = mybir.dt
    # qt+kt merged, per duo: [kt-half0 | qt-half0 | kt-half1 | qt-half1]
    qk = nc.dram_tensor("qk", [2 * C, (PAIRS // 2) * 2 * S], dt.bfloat16, kind="ExternalInput").ap()
    va = nc.dram_tensor("va", [T, PAIRS * NT * VW], dt.bfloat16, kind="ExternalInput").ap()

    # flatten (duo, group) sequence; chunk eb DMA every CHUNK groups
    GL = []  # (duo, tiles, eb_offset_cols)
    eboff = 0
    for du in range(PAIRS // 2):
        for g in groups[du]:
            GL.append((du, g, eboff))
            eboff += 2 * len(g) * T
    n = len(GL)
    bounds = [0, min(2, n)]  # small first chunk so the first mul lands early
    while bounds[-1] < n:
        bounds.append(min(bounds[-1] + CHUNK, n))
    CH = []  # chunk -> (start col, n cols)
    chunk_of = [0] * n
    for ci in range(len(bounds) - 1):
        g0, g1 = bounds[ci], bounds[ci + 1]
        lo = GL[g0][2]
        hi = GL[g1][2] if g1 < n else eboff
        CH.append((lo, hi - lo))
        for g in range(g0, g1):
            chunk_of[g] = ci
    nch = len(CH)

    eb = nc.dram_tensor("eb", [T, eboff], dt.bfloat16, kind="ExternalInput").ap()
    o = nc.dram_tensor("o", [T, PAIRS, NT * VW], dt.bfloat16, kind="ExternalOutput").ap()
    CAP = GCAP * T

    with _FastTailTile(nc) as tc:
        with (
            tc.tile_pool(name="res", bufs=1) as res,
            tc.tile_pool(name="io", bufs=8) as io,
            tc.tile_pool(name="wk", bufs=6) as wk,
            tc.tile_pool(name="ob", bufs=4) as obp,
            tc.tile_pool(name="ops", bufs=2, space="PSUM") as ops,
            tc.tile_pool(name="lps", bufs=2, space="PSUM") as lps,
        ):
            # Warm the ScalarE Exp spline table during the DMA preamble:
            # walrus loads the ACT table set at the first ACTIVATE (~2.7us),
            # which otherwise lands on the first group's critical chain.
            hoist = []
            warm = res.tile([T, 1], dt.float32, tag="actwarm")
            hoist.append(nc.gpsimd.memset(warm[:], 0.0).ins)
            hoist.append(nc.scalar.activation(
                warm[:], warm[:], mybir.ActivationFunctionType.Exp).ins)

            qt_sb, va_sb = {}, {}
            st = {}  # g -> dict of live tiles
            chst = {}  # chunk -> eb tile
            o_ps = {}  # half -> current psum out block
            obt_cur = {}  # block -> staging tile

            def fetch_chunk(c):
                if c in chst or c >= nch:
                    return
                lo, cols = CH[c]
                ebt = io.tile([T, cols], dt.bfloat16, tag="eb", name=f"ebc{c}")
                # c0 rides the sync HWDGE ring; the next two (needed by
                # ~13us, before SWDGE wakes up) ride the scalar HWDGE ring
                # whose queue is idle until the first real exp; the rest use
                # the gpsimd SWDGE queue.
                if c == 0:
                    eng = nc.sync
                elif c <= 2:
                    eng = nc.scalar
                else:
                    eng = nc.gpsimd
                di = eng.dma_start(ebt[:], eb[:, lo:lo + cols])
                if c <= 2:
                    hoist.append(di.ins)
                chst[c] = (ebt, lo)

            def alloc_duo(du):
                pA, pB = 2 * du, 2 * du + 1
                qt_sb[du] = res.tile([2 * C, 2 * S], dt.bfloat16, tag=f"qk{du}", name=f"qk{du}")
                vduo = res.tile([T, 2 * NT * VW], dt.bfloat16, tag=f"va{du}", name=f"vad{du}")
                va_sb[pA] = vduo[:, 0:NT * VW]
                va_sb[pB] = vduo[:, NT * VW:2 * NT * VW]
                return vduo

            # prologue: the sync (HWDGE) queue carries the resident inputs
            # plus eb chunk0, ordered by first use, hoisted ahead of the
            # bass preamble; later eb chunks ride the gpsimd (SWDGE) queue
            # from the body (a hoisted SWDGE DMA would stall the Pool
            # preamble drain and with it the whole entry barrier).
            v0 = alloc_duo(0)
            v1 = alloc_duo(1)
            hoist.append(nc.sync.dma_start(qt_sb[0][:, 0:S], qk[:, 0:S]).ins)
            fetch_chunk(0)
            hoist.append(nc.sync.dma_start(v0[:], va[:, 0:2 * NT * VW]).ins)
            hoist.append(nc.sync.dma_start(qt_sb[0][:, S:2 * S], qk[:, S:2 * S]).ins)
            hoist.append(nc.sync.dma_start(qt_sb[1][:], qk[:, 2 * S:4 * S]).ins)
            hoist.append(nc.sync.dma_start(v1[:], va[:, 2 * NT * VW:4 * NT * VW]).ins)
            for _c in range(1, nch):
                fetch_chunk(_c)

            HS = S // 2

            def ktc(s0):
                return s0 if s0 < HS else s0 + HS

            def qtc(s0):
                return s0 + HS if s0 < HS else s0 + 2 * HS

            def stage_a(g):
                du, tg, off = GL[g]
                cols = len(tg) * T
                l_ps = lps.tile([T, 2 * CAP], dt.float32, tag="l", name=f"l{g}")
                for idx, (i, j) in enumerate(tg):
                    for h, c0 in ((0, 0), (C, CAP)):
                        nc.tensor.matmul(
                            l_ps[:, c0 + idx * T:c0 + (idx + 1) * T],
                            qt_sb[du][h:h + C, ktc(j * T):ktc(j * T) + T],
                            qt_sb[du][h:h + C, qtc(i * T):qtc(i * T) + T],
                            start=True, stop=True, skip_group_check=True,
                        )
                ebt, base = chst[chunk_of[g]]
                st[g] = dict(eb=ebt[:, off - base:off - base + 2 * cols], l=l_ps)

            def stage_b(g):
                du, tg, off = GL[g]
                cols = len(tg) * T
                el_sb = wk.tile([T, 2 * CAP], dt.bfloat16, tag="el", name=f"el{g}")
                nc.scalar.activation(el_sb[:, 0:CAP + cols],
                                     st[g]["l"][:, 0:CAP + cols],
                                     mybir.ActivationFunctionType.Exp)
                w_sb = wk.tile([T, 2 * cols], dt.bfloat16, tag="w", name=f"w{g}")
                if cols == CAP:
                    nc.vector.tensor_mul(w_sb[:], el_sb[:], st[g]["eb"])
                else:
                    nc.vector.tensor_mul(w_sb[:, 0:cols], el_sb[:, 0:cols],
                                         st[g]["eb"][:, 0:cols])
                    nc.vector.tensor_mul(w_sb[:, cols:2 * cols],
                                         el_sb[:, CAP:CAP + cols],
                                         st[g]["eb"][:, cols:2 * cols])
                st[g]["w"] = w_sb

            def stage_c(g):
                du, tg, off = GL[g]
                ks = kstart[du]
                w_sb = st[g]["w"]
                cols = len(tg) * T
                for half, p in ((0, 2 * du), (1, 2 * du + 1)):
                    for idx, (i, j) in enumerate(tg):
                        if j == ks[i] and i % OUT_BLK == 0:
                            o_ps[half] = ops.tile([T, OUT_BLK * VW], dt.float32,
                                                  tag=f"o{half}", name=f"o{half}_{g}_{i}")
                        t_ = i % OUT_BLK
                        nc.tensor.matmul(
                            o_ps[half][:, t_ * VW:(t_ + 1) * VW],
                            w_sb[:, half * cols + idx * T:half * cols + (idx + 1) * T],
                            va_sb[p][:, j * VW:(j + 1) * VW],
                            start=(j == ks[i]), stop=(j == i),
                            skip_group_check=True,
                        )
                        if j == i and i % OUT_BLK == OUT_BLK - 1:
                            blk = i // OUT_BLK
                            if half == 0:
                                obt_cur[blk] = obp.tile(
                                    [T, 2 * OUT_BLK * VW], dt.bfloat16,
                                    tag="obt", name=f"ob{g}_{i}")
                            obt = obt_cur[blk]
                            bw = OUT_BLK * VW
                            nc.vector.tensor_copy(
                                obt[:, half * bw:(half + 1) * bw], o_ps[half][:])
                            if half == 1:
                                c0 = blk * bw
                                nc.sync.dma_start(
                                    o[:, 2 * du:2 * du + 2, c0:c0 + bw], obt[:])
                del st[g]

            for g in range(n + 2):
                if 0 <= g - 1 < n:
                    stage_b(g - 1)
                if g < n:
                    stage_a(g)
                if g - 2 >= 0:
                    stage_c(g - 2)
    if HOIST:
        f = nc.m.functions[0]
        movable = [i for i in hoist
                   if not (i.sync_info and i.sync_info.on_wait)]
        names = {i.name for i in movable}
        for bb in f.blocks[1:]:
            kept = [i for i in bb.instructions if i.name not in names]
            if len(kept) != len(bb.instructions):
                bb.instructions = kept
        b0 = f.blocks[0]
        cur = b0.instructions
        b0.instructions = cur[:1] + movable + cur[1:]
    nc.compile()
    return nc


def _stage_inputs(q, k, v, b, m, groups):
    """Build per-core in_maps (host-side transposes, exp(bias)*mask, packing)."""
    ebtot = 2 * sum(len(g) for pg in groups for g in pg)
    masks = []
    for b_ in range(B):
        seg = m[b_][:, None] == m[b_][None, :]
        causal = np.tri(S, S, 0, dtype=bool)
        masks.append(seg & causal)

    ones = np.ones((S, 1), np.float32)
    in_maps = []
    for core in range(NCORE):
        qk = np.empty((2 * C, (PAIRS // 2) * 2 * S), bf16)
        HS = S // 2
        va = np.empty((T, PAIRS * NT * VW), bf16)
        ebp = np.empty((T, ebtot * T), bf16)
        E = {}
        for p in range(PAIRS):
            b_, h = p // HPC, HPC * core + p % HPC
            du, half = p // 2, p % 2
            base = du * 2 * S
            qT = (q[b_, :, h, :].T * SM).astype(bf16)
            kT = k[b_, :, h, :].T.astype(bf16)
            r0, r1 = half * C, (half + 1) * C
            qk[r0:r1, base:base + HS] = kT[:, 0:HS]
            qk[r0:r1, base + HS:base + 2 * HS] = qT[:, 0:HS]
            qk[r0:r1, base + 2 * HS:base + 3 * HS] = kT[:, HS:S]
            qk[r0:r1, base + 3 * HS:base + 4 * HS] = qT[:, HS:S]
            vv = np.concatenate([v[b_, :, h, :], ones], 1).astype(bf16)
            va[:, p * NT * VW:(p + 1) * NT * VW] = (
                vv.reshape(NT, T, VW).transpose(1, 0, 2).reshape(T, NT * VW))
            E[p] = np.exp(b[b_, h].astype(np.float32))
        eboff = 0
        for du in range(PAIRS // 2):
            Mk = masks[du]
            for g in groups[du]:
                for p in (2 * du, 2 * du + 1):
                    for (i, j) in g:
                        blk = np.where(Mk[i * T:(i + 1) * T, j * T:(j + 1) * T].T,
                                       E[p][i * T:(i + 1) * T, j * T:(j + 1) * T].T, 0.0)
                        ebp[:, eboff:eboff + T] = blk.astype(bf16)
                        eboff += T
        assert eboff == ebtot * T
        in_maps.append({"qk": qk, "va": va, "eb": ebp})
    return in_maps


def _unstage(results):
    """results[c]["o"] [T, PAIRS*NT*VW] f32 -> out [B, S, H, C] f32."""
    out = np.empty((B, S, H, C), np.float32)
    for core in range(NCORE):
        oc = np.asarray(results[core]["o"]).astype(np.float32).reshape(T, PAIRS, NT * VW)
        for p in range(PAIRS):
            b_, h = p // HPC, HPC * core + p % HPC
            blk = oc[:, p, :].reshape(T, NT, VW)
            blk = blk.transpose(1, 0, 2).reshape(S, VW)
            out[b_, :, h, :] = blk[:, :C] / blk[:, C:]
    return out


_CACHE = {}


def _get_nc(groups_key, kstart, groups):
    if groups_key not in _CACHE:
        _CACHE[groups_key] = _build(kstart, groups)
    return _CACHE[groups_key]


def kernel(q, k, v, b, m, _trace=False, _trace_cores=None):
    q = np.asarray(q, np.float32)
    k = np.asarray(k, np.float32)
    v = np.asarray(v, np.float32)
    b = np.asarray(b, np.float32)
    m = np.asarray(m)
    kstart, groups = _plan(m)
    groups_key = str(groups)
    nc = _get_nc(groups_key, kstart, groups)
    in_maps = _stage_inputs(q, k, v, b, m, groups)
    res = None
    for attempt in range(3):
        try:
            res = run_bass_kernel_spmd(nc, in_maps, core_ids=list(range(NCORE)),
                                       trace=_trace, trace_cores=_trace_cores)
            break
        except Exception:
            if attempt == 2:
                raise
    out = _unstage(res.results)
    kernel.last_results = res
    return out


if __name__ == "__main__":
    rng = np.random.default_rng(0)
    q = rng.standard_normal((B, S, H, C), np.float32)
    k = rng.standard_normal((B, S, H, C), np.float32)
    v = rng.standard_normal((B, S, H, C), np.float32)
    bb = rng.standard_normal((B, H, S, S), np.float32)
    mm = np.sort(rng.integers(0, 4, (B, S)).astype(np.int32), -1)
    o = kernel(q, k, v, bb, mm)
    print("kernel ran, out shape", o.shape, "finite:", np.isfinite(o).all())
